# revision 1
# baseline (speedup 1.0000x reference)
"""CrossAttentionBlock kernel for 8 trn2 NeuronCores.

Sharding: core c = b*4 + hg handles batch b (of 2) and head-group hg
(4 of the 16 heads, a contiguous 256-wide slice of the 1024 channel dim).
Each core computes its partial output projection; the host sums the 4
partials per batch and adds bproj. No cross-core communication.

Per-core pipeline (all matmuls float32r, 1 cyc/row at N>=256):
  kT[d,m] = Wk_slice @ y^T           (d-major, lhsT=WkT chunks, rhs=yT)
  v[m, d] (65-col blocks per head, ones column rides along for softmax sums)
  qT[d,n] likewise from x
  LayerNorm over d (partition dim) via matmul partition-reductions and
  gamma-folded broadcast matmuls.
  ST[m,n] = kT.T @ qT per head (two heads row-packed in the PE array),
  PT = exp(ST)  (max-free softmax: |scores| <= 8 after LN), then
  OT[d,n] += V_ext.T @ PT  -- row 64 of OT = softmax sums (ones column).
  Normalize OT by broadcast(1/sums), project: out += OT.T @ WprojT.
"""

import sys

import numpy as np

if "/opt/trn_rl_repo" not in sys.path:
    sys.path.insert(0, "/opt/trn_rl_repo")

import concourse.bacc as bacc
import concourse.tile as tile
from concourse import mybir
from concourse.bass_utils import run_bass_kernel_spmd

F32 = mybir.dt.float32
F32R = mybir.dt.float32r
AF = mybir.ActivationFunctionType
ALU = mybir.AluOpType

C = 1024          # model dim
NT = 2048         # sequence length (N == M)
HD = 64           # head dim
NHL = 4           # heads per core
DL = NHL * HD     # 256 local channel width
P = 128           # partitions
CH = 512          # n-chunk
NCH = NT // CH    # 4 chunks
MTILES = NT // P  # 16 m-tiles
SCALE = HD ** -0.5
LN_EPS = 1e-5
VW = HD + 1       # 65: v block per head: [v(64), ones column]

_CACHED = None


def _build(chain=1):
    nc = bacc.Bacc()

    xT = nc.declare_dram_parameter("xT", [C, NT], F32R, isOutput=False)
    yT = nc.declare_dram_parameter("yT", [C, NT], F32R, isOutput=False)
    wqT = nc.declare_dram_parameter("wqT", [C, DL], F32R, isOutput=False)
    wkT = nc.declare_dram_parameter("wkT", [C, DL], F32R, isOutput=False)
    wvT = nc.declare_dram_parameter("wvT", [C, NHL * VW], F32R, isOutput=False)
    wpT = nc.declare_dram_parameter("wpT", [DL, C], F32R, isOutput=False)
    # packed constants: cblob cols = [bqc(2), bkc(2), betaq, betak, gq, gk,
    # eps]; rowblob = [one1(P) | bvr_r]; selblob = [selA | selB]
    cblob = nc.declare_dram_parameter("cblob", [P, 9], F32, isOutput=False)
    osel = nc.declare_dram_parameter("osel", [P, P], F32R, isOutput=False)
    selblob = nc.declare_dram_parameter("selblob", [65, 2 * P], F32R,
                                        isOutput=False)
    rowblob = nc.declare_dram_parameter("rowblob", [1, P + NHL * VW], F32R,
                                        isOutput=False)
    out = nc.declare_dram_parameter("out", [NT, C], F32, isOutput=True)

    from contextlib import ExitStack

    with tile.TileContext(nc) as tc:
      for _rep in range(chain):
       with ExitStack() as top:
        cp = top.enter_context(tc.tile_pool(name="const", bufs=1))
        t_cblob = cp.tile([P, 9], F32)
        t_osel = cp.tile([P, P], F32R)
        t_selblob = cp.tile([65, 2 * P], F32R)
        t_rowblob = cp.tile([1, P + NHL * VW], F32R)
        t_bqc = t_cblob[:, 0:2]
        t_bkc = t_cblob[:, 2:4]
        t_betaq = t_cblob[:, 4:5]
        t_betak = t_cblob[:, 5:6]
        t_gq = t_cblob[:, 6:7]
        t_gk = t_cblob[:, 7:8]
        t_eps = t_cblob[:, 8:9]
        t_selA = t_selblob[:, 0:P]
        t_selB = t_selblob[:, P:2 * P]
        t_one1 = t_rowblob[:, 0:P]
        t_bvr_r = t_rowblob[:, P:P + NHL * VW]

        def _load_consts():
            nc.sync.dma_start(t_cblob[:], cblob[:])
            nc.sync.dma_start(t_osel[:], osel[:])
            nc.sync.dma_start(t_rowblob[:], rowblob[:])

        def _load_sel():
            nc.sync.dma_start(t_selblob[:], selblob[:])

        pp = top.enter_context(tc.tile_pool(name="persist", bufs=1))
        kT_ln = [pp.tile([P, NT], F32R, tag=f"kTln{i}", name=f"kTln{i}") for i in range(2)]
        qT_ln = [pp.tile([P, NT], F32R, tag=f"qTln{i}", name=f"qTln{i}") for i in range(2)]
        v_sb = pp.tile([P, MTILES * NHL * VW], F32R, tag="v", name="v_sb")
        ot_sb = [pp.tile([P, NT], F32, tag=f"ot{i}", name=f"ot{i}") for i in range(2)]
        otn = [pp.tile([P, NT], F32R, tag=f"otn{i}", name=f"otn{i}") for i in range(2)]
        wp_sb = [pp.tile([P, C], F32R, tag=f"wp{i}", name=f"wp{i}") for i in range(2)]

        def proj_phase(src3, wT_sb, bias_col, gcol, bcol, lnout,
                       do_v, ablock_pool, mm_ps, v_ps, st_ps, sc_pool,
                       after_first_block=None):
            """Stream src (xT or yT) in column blocks; d-major projection
            (+bias on ACT), fused LayerNorm (broadcast-stats via osel
            matmuls, smalls split across ACT/DVE) into lnout, optionally v.
            LN runs on [128, 1024] half-d-tile chunks."""
            for ch in range(NCH):
                yt = ablock_pool.tile([P, 8 * CH], F32R, tag="ablock",
                                      name="ablock")
                nc.sync.dma_start(
                    yt[:].rearrange("p (c n) -> p c n", n=CH),
                    src3[:, :, ch * CH:(ch + 1) * CH],
                )
                if ch == 0 and after_first_block is not None:
                    after_first_block()
                for dt in range(2):
                    sl = slice(ch * CH, (ch + 1) * CH)
                    raw = sc_pool.tile([P, CH], F32R, tag="raw", name="raw",
                                       bufs=3)
                    ps = mm_ps.tile([P, CH], F32, tag="mmps", name="mmps")
                    for ct in range(8):
                        nc.tensor.matmul(
                            ps[:],
                            wT_sb[:, ct * DL + dt * P: ct * DL + (dt + 1) * P],
                            yt[:, ct * CH:(ct + 1) * CH],
                            start=(ct == 0), stop=(ct == 7),
                        )
                    nc.scalar.add(raw[:], ps[:], bias_col[:, dt:dt + 1])
                    sq = sc_pool.tile([P, CH], F32R, tag="sq", name="sq",
                                      bufs=2)
                    nc.scalar.activation(sq[:], raw[:], AF.Square)
                    mean_ps = st_ps.tile([P, CH], F32, tag="meanps",
                                         name="meanps")
                    nc.tensor.matmul(mean_ps[:], t_osel[:], raw[:],
                                     start=True, stop=True)
                    msq_ps = st_ps.tile([P, CH], F32, tag="msqps",
                                        name="msqps")
                    nc.tensor.matmul(msq_ps[:], t_osel[:], sq[:],
                                     start=True, stop=True)
                    # LN smalls, [128, CH] partition-dense, ACT/DVE balanced
                    t1 = sc_pool.tile([P, CH], F32, tag="t1", name="t1", bufs=2)
                    nc.scalar.activation(t1[:], mean_ps[:], AF.Square)
                    var = sc_pool.tile([P, CH], F32, tag="var", name="var", bufs=2)
                    nc.vector.tensor_sub(var[:], msq_ps[:], t1[:])
                    sd = sc_pool.tile([P, CH], F32, tag="sd", name="sd", bufs=2)
                    nc.scalar.activation(sd[:], var[:], AF.Sqrt,
                                         bias=t_eps[:, 0:1])
                    rstd = sc_pool.tile([P, CH], F32, tag="rstd", name="rstd", bufs=2)
                    nc.vector.reciprocal_approx_fast(rstd[:], sd[:])
                    rg = sc_pool.tile([P, CH], F32, tag="rg", name="rg", bufs=2)
                    nc.scalar.mul(rg[:], rstd[:], gcol[:, 0:1])
                    tq = sc_pool.tile([P, CH], F32, tag="tq", name="tq", bufs=2)
                    nc.vector.tensor_mul(tq[:], raw[:], rg[:])
                    b0 = sc_pool.tile([P, CH], F32, tag="b0", name="b0", bufs=2)
                    nc.vector.tensor_mul(b0[:], mean_ps[:], rg[:])
                    nc.vector.scalar_tensor_tensor(
                        lnout[dt][:, sl], tq[:], bcol[:, 0:1], b0[:],
                        ALU.add, ALU.subtract)
                if do_v:
                    for j in range(4):
                        vp = v_ps.tile([P, NHL * VW], F32, tag="vps",
                                       name="vps")
                        for ct in range(8):
                            nc.tensor.matmul(
                                vp[:],
                                yt[:, ct * CH + j * P: ct * CH + (j + 1) * P],
                                wvT_sb[:, ct * NHL * VW:(ct + 1) * NHL * VW],
                                start=(ct == 0), stop=False,
                            )
                        nc.tensor.matmul(
                            vp[:], t_one1[0:1, 0:P], t_bvr_r[0:1, :],
                            start=False, stop=True)
                        m = 4 * ch + j
                        nc.scalar.copy(
                            v_sb[:, m * NHL * VW:(m + 1) * NHL * VW], vp[:])

        # ---------------- projections: K/V then Q (shared pools) ----------------
        with ExitStack() as ph:
            wpool = ph.enter_context(tc.tile_pool(name="wkv", bufs=1))
            wkT_sb = wpool.tile([P, 8 * DL], F32R)
            wvT_sb = wpool.tile([P, 8 * NHL * VW], F32R)
            wqT_sb = wpool.tile([P, 8 * DL], F32R)
            nc.sync.dma_start(
                wkT_sb[:].rearrange("p (c d) -> p c d", d=DL),
                wkT[:].rearrange("(c p) d -> p c d", p=P))
            def _load_wv():
                _load_consts()
                nc.sync.dma_start(
                    wvT_sb[:].rearrange("p (c d) -> p c d", d=NHL * VW),
                    wvT[:].rearrange("(c p) d -> p c d", p=P))
                _load_sel()
            ablock = ph.enter_context(tc.tile_pool(name="ablk", bufs=3))
            sc_pool = ph.enter_context(tc.tile_pool(name="sc", bufs=1))
            mm_ps = ph.enter_context(
                tc.tile_pool(name="mmps", bufs=3, space="PSUM"))
            v_ps = ph.enter_context(
                tc.tile_pool(name="vps", bufs=1, space="PSUM"))
            st_ps = ph.enter_context(
                tc.tile_pool(name="stps", bufs=2, space="PSUM"))
            y3 = yT[:].rearrange("(c p) n -> p c n", p=P)
            x3 = xT[:].rearrange("(c p) n -> p c n", p=P)
            proj_phase(y3, wkT_sb, t_bkc, t_gk, t_betak, kT_ln, True,
                       ablock, mm_ps, v_ps, st_ps, sc_pool,
                       after_first_block=_load_wv)
            nc.sync.dma_start(
                wqT_sb[:].rearrange("p (c d) -> p c d", d=DL),
                wqT[:].rearrange("(c p) d -> p c d", p=P))
            nc.sync.dma_start(wp_sb[0][:], wpT[0:P, :])
            nc.sync.dma_start(wp_sb[1][:], wpT[P:DL, :])
            proj_phase(x3, wqT_sb, t_bqc, t_gq, t_betaq, qT_ln, False,
                       ablock, mm_ps, None, st_ps, sc_pool)

        # ---------------- attention + projection ----------------
        with ExitStack() as ph:
            stp = ph.enter_context(
                tc.tile_pool(name="stattn", bufs=2, space="PSUM"))
            otp = ph.enter_context(
                tc.tile_pool(name="otps", bufs=2, space="PSUM"))
            smallp = ph.enter_context(
                tc.tile_pool(name="smallps", bufs=2, space="PSUM"))
            ptp = ph.enter_context(tc.tile_pool(name="pt", bufs=3))
            rcp = ph.enter_context(tc.tile_pool(name="rcp", bufs=2))
            outp = ph.enter_context(tc.tile_pool(name="outsb", bufs=2))

            def emit_norm(p, ch, stgA, stgB):
                sl = slice(ch * CH, (ch + 1) * CH)
                bc = smallp.tile([P, CH], F32, tag="smallps", name="bcn")
                nc.tensor.matmul(bc[:], t_selA[64:65, 0:P], stgA[64:65, :],
                                 start=True, stop=False)
                nc.tensor.matmul(bc[:], t_selB[64:65, 0:P], stgB[64:65, :],
                                 start=False, stop=True)
                rb = rcp.tile([P, CH], F32, tag="rb", name="rb")
                nc.vector.reciprocal_approx_fast(rb[:], bc[:])
                nc.vector.tensor_mul(otn[p][:, sl], ot_sb[p][:, sl], rb[:])

            def emit_proj(ch):
                for j in range(4):
                    ntile = ch * 4 + j
                    ob = outp.tile([P, C], F32, tag="outsb", name="ob")
                    for cc in range(2):
                        pj = smallp.tile([P, CH], F32, tag="smallps",
                                         name="pj")
                        nc.tensor.matmul(
                            pj[:], otn[0][:, ntile * P:(ntile + 1) * P],
                            wp_sb[0][:, cc * CH:(cc + 1) * CH],
                            start=True, stop=False)
                        nc.tensor.matmul(
                            pj[:], otn[1][:, ntile * P:(ntile + 1) * P],
                            wp_sb[1][:, cc * CH:(cc + 1) * CH],
                            start=False, stop=True)
                        nc.vector.tensor_copy(ob[:, cc * CH:(cc + 1) * CH],
                                              pj[:])
                    nc.sync.dma_start(out[ntile * P:(ntile + 1) * P, :],
                                      ob[:])

            pending = None
            stages = {}
            for ch in range(NCH):
                sl = slice(ch * CH, (ch + 1) * CH)
                for p in range(2):
                    otA = otp.tile([P, CH], F32, tag="otps", name="otA")
                    otB = otp.tile([P, CH], F32, tag="otps", name="otB")
                    for m in range(MTILES):
                        st = stp.tile([P, 2 * CH], F32, name="st")
                        nc.tensor.matmul(
                            st[:, 0:CH],
                            kT_ln[p][0:HD, m * P:(m + 1) * P],
                            qT_ln[p][0:HD, sl],
                            start=True, stop=True, tile_position=(0, 0))
                        nc.tensor.matmul(
                            st[:, CH:2 * CH],
                            kT_ln[p][HD:P, m * P:(m + 1) * P],
                            qT_ln[p][HD:P, sl],
                            start=True, stop=True, tile_position=(64, 0))
                        pt = ptp.tile([P, 2 * CH], F32R, name="pt")
                        nc.scalar.activation(pt[:], st[:], AF.Exp)
                        base = m * NHL * VW
                        nc.tensor.matmul(
                            otA[0:VW, :],
                            v_sb[:, base + 2 * p * VW: base + (2 * p + 1) * VW],
                            pt[:, 0:CH],
                            start=(m == 0), stop=(m == MTILES - 1))
                        nc.tensor.matmul(
                            otB[0:VW, :],
                            v_sb[:, base + (2 * p + 1) * VW: base + (2 * p + 2) * VW],
                            pt[:, CH:2 * CH],
                            start=(m == 0), stop=(m == MTILES - 1))
                    nc.vector.tensor_copy(ot_sb[p][0:HD, sl], otA[0:HD, :])
                    nc.vector.tensor_copy(ot_sb[p][HD:P, sl], otB[0:HD, :])
                    stgA = rcp.tile([65, CH], F32R, tag="stgA", name="stgA")
                    stgB = rcp.tile([65, CH], F32R, tag="stgB", name="stgB")
                    nc.vector.tensor_copy(stgA[64:65, :], otA[64:65, :])
                    nc.vector.tensor_copy(stgB[64:65, :], otB[64:65, :])
                    stages[p] = (stgA, stgB)
                    if p == 0 and pending is not None:
                        pch = pending
                        emit_norm(1, pch, *stages_prev)
                        emit_proj(pch)
                        pending = None
                    if p == 1:
                        emit_norm(0, ch, *stages[0])
                pending = ch
                stages_prev = stages[1]
                stages = {}
            emit_norm(1, pending, *stages_prev)
            emit_proj(pending)

    nc.finalize()
    return nc


def _get_nc():
    global _CACHED
    if _CACHED is None:
        _CACHED = _build()
    return _CACHED


def _host_inputs(x, y, Wq, bq, Wkv, bkv, q_gamma, q_beta, k_gamma, k_beta,
                 Wproj, bproj):
    f = np.float32
    in_maps = []
    for c in range(8):
        b, hg = divmod(c, 4)
        hs = hg * DL
        xT = np.ascontiguousarray(x[b].T, dtype=f)
        yT = np.ascontiguousarray(y[b].T, dtype=f)
        wqT = np.ascontiguousarray(Wq[hs:hs + DL].T, dtype=f)
        wkT = np.ascontiguousarray(Wkv[hs:hs + DL].T, dtype=f)
        Wv_s = Wkv[C + hs: C + hs + DL]
        wvT = np.zeros((C, NHL * VW), dtype=f)
        bvr_r = np.zeros((1, NHL * VW), dtype=f)
        bv_s = bkv[C + hs: C + hs + DL]
        for h in range(NHL):
            wvT[:, h * VW:h * VW + HD] = Wv_s[h * HD:(h + 1) * HD].T
            bvr_r[0, h * VW:h * VW + HD] = bv_s[h * HD:(h + 1) * HD]
            bvr_r[0, h * VW + HD] = 1.0
        wpT = np.ascontiguousarray(Wproj[:, hs:hs + DL].T, dtype=f)
        cblob = np.zeros((P, 9), dtype=f)
        cblob[:, 0] = bq[hs:hs + P]
        cblob[:, 1] = bq[hs + P:hs + DL]
        cblob[:, 2] = bkv[hs:hs + P]
        cblob[:, 3] = bkv[hs + P:hs + DL]
        cblob[:, 4] = np.tile(q_beta * SCALE, 2)
        cblob[:, 5] = np.tile(k_beta, 2)
        cblob[:, 6] = np.tile(q_gamma * SCALE, 2)
        cblob[:, 7] = np.tile(k_gamma, 2)
        cblob[:, 8] = LN_EPS
        selblob = np.zeros((65, 2 * P), dtype=f)
        selblob[64, 0:HD] = 1.0
        selblob[64, P + HD:2 * P] = 1.0
        osel = np.zeros((P, P), dtype=f)
        osel[0:HD, 0:HD] = 1.0 / HD
        osel[HD:P, HD:P] = 1.0 / HD
        rowblob = np.zeros((1, P + NHL * VW), dtype=f)
        rowblob[0, 0:P] = 1.0
        rowblob[0, P:] = bvr_r[0]
        in_maps.append({
            "xT": xT, "yT": yT, "wqT": wqT, "wkT": wkT, "wvT": wvT,
            "wpT": wpT, "cblob": cblob, "osel": osel, "selblob": selblob,
            "rowblob": rowblob,
        })
    return in_maps


def kernel(x, y, Wq, bq, Wkv, bkv, q_gamma, q_beta, k_gamma, k_beta,
           Wproj, bproj, _trace=False, _trace_kwargs=None):
    args = [np.asarray(a, dtype=np.float32)
            for a in (x, y, Wq, bq, Wkv, bkv, q_gamma, q_beta, k_gamma,
                      k_beta, Wproj, bproj)]
    (x, y, Wq, bq, Wkv, bkv, q_gamma, q_beta, k_gamma, k_beta,
     Wproj, bproj) = args
    nc = _get_nc()
    in_maps = _host_inputs(x, y, Wq, bq, Wkv, bkv, q_gamma, q_beta,
                           k_gamma, k_beta, Wproj, bproj)
    kw = {}
    if _trace:
        kw = {"trace": True, **(_trace_kwargs or {})}
    res = run_bass_kernel_spmd(nc, in_maps, list(range(8)), **kw)
    B = x.shape[0]
    out = np.zeros((B, NT, C), dtype=np.float32)
    for c in range(8):
        b = c // 4
        out[b] += res.results[c]["out"]
    out += bproj[None, None, :]
    if _trace:
        return out, res
    return out

